# revision 1
# baseline (speedup 1.0000x reference)
"""Fused transformer block (attention + MLP) on 8 trn2 NeuronCores.

Sharding (8-way, batch-symmetric): every core computes attention for ONE
head-pair (heads 2c, 2c+1) of BOTH batches, and owns a 256-token shard of
each batch (tokens [256c, 256c+256)). The two shard halves are concatenated
along the free axis into one 512-column tile set, so projection + FFN code
is identical to a single 512-token shard.

Exchange: after each batch's attention finishes, one 8-way bf16 AllToAll
(512KB) redistributes the attnT head-rows by destination token shard. The
batch-0 AllToAll runs concurrently with batch-1 attention compute, so only
the batch-1 exchange (~10us) is exposed. Each core then computes the full
attention projection for its own shard locally (it holds the full Wproj),
followed by the full FFN (sequence parallel).

On-chip layout is "T-last" (feature dim on partitions, tokens on the free
axis) so no transposes are ever needed. V is built in natural [T, hs]
layout with a ones-column appended so the softmax denominator falls out of
the same matmul. The softmax 1/den broadcast is done with a tensor-engine
outer product (ones[1,64]^T @ rc[1,512]) instead of gpsimd
partition_broadcast, keeping the gpsimd queue free for the collective wait.
Logits are tiny (|s| < ~1), so softmax needs no max subtraction. A short
warm-up burst of matmuls on a memset tile runs during the initial input
DMA so the PE clock is at 2.4 GHz when real work starts.
"""

import sys

for _p in ("/opt/trn_rl_repo",):
    if _p not in sys.path:
        sys.path.append(_p)

import numpy as np
import ml_dtypes

import concourse.bass as bass
import concourse.tile as tile
from concourse import bacc, mybir
from concourse.bass_utils import run_bass_kernel_spmd

BF16 = mybir.dt.bfloat16
F32 = mybir.dt.float32
AF = mybir.ActivationFunctionType
ALU = mybir.AluOpType

N_CORES = 8
B, T, C = 2, 2048, 1024
H, HS = 16, 64
F = 4 * C
TS = 512          # per-core fused shard width (256 tokens x 2 batches)
TSB = 256         # per-batch shard width
CC = C // 128
FB = F // 128
SCALE = float(C) ** -0.5


def build_program(nc: bass.Bass):
    xt_bf = nc.dram_tensor("xt_bf", [B, CC, 128, T], BF16,
                           kind="ExternalInput").ap()
    xts_f = nc.dram_tensor("xts_f", [CC, 128, TS], F32, kind="ExternalInput").ap()
    wq_d = nc.dram_tensor("wq", [CC, 128, 128], BF16, kind="ExternalInput").ap()
    wk_d = nc.dram_tensor("wk", [CC, 128, 128], BF16, kind="ExternalInput").ap()
    wv_d = nc.dram_tensor("wv", [CC, 128, 128], BF16, kind="ExternalInput").ap()
    wp_d = nc.dram_tensor("wp", [8, CC, 128, 128], BF16,
                          kind="ExternalInput").ap()
    w1_d = nc.dram_tensor("w1", [CC, 128, F], BF16, kind="ExternalInput").ap()
    w2_d = nc.dram_tensor("w2", [FB, 128, C], BF16, kind="ExternalInput").ap()
    bp_d = nc.dram_tensor("bp", [CC, 128, 1], F32, kind="ExternalInput").ap()
    b1_d = nc.dram_tensor("b1", [FB, 128, 1], F32, kind="ExternalInput").ap()
    b2_d = nc.dram_tensor("b2", [CC, 128, 1], F32, kind="ExternalInput").ap()
    out_d = nc.dram_tensor("outT", [CC, 128, TS], F32, kind="ExternalOutput").ap()

    with tile.TileContext(nc) as tc:
        _emit(nc, tc, xt_bf, xts_f, wq_d, wk_d, wv_d, wp_d, w1_d, w2_d,
              bp_d, b1_d, b2_d, out_d)


def _emit(nc, tc, xt_bf, xts_f, wq_d, wk_d, wv_d, wp_d, w1_d, w2_d,
          bp_d, b1_d, b2_d, out_d):
    from contextlib import ExitStack

    ctx = ExitStack()
    with ctx:
        st = ctx.enter_context(tc.tile_pool(name="static", bufs=1))
        big = ctx.enter_context(tc.tile_pool(name="big", bufs=12))
        w2p = ctx.enter_context(tc.tile_pool(name="w2s", bufs=3))
        expp = ctx.enter_context(tc.tile_pool(name="expp", bufs=6))
        outp = ctx.enter_context(tc.tile_pool(name="outp", bufs=2))
        rcp = ctx.enter_context(tc.tile_pool(name="rcp", bufs=2))
        w2b = ctx.enter_context(tc.tile_pool(name="w2b", bufs=4))

        ps_ctx = tc.tile_pool(name="ps", bufs=2, space="PSUM")
        ps = ps_ctx.__enter__()
        attn_psum = tc.tile_pool(name="apsum", bufs=2, space="PSUM")
        psc_pool = attn_psum.__enter__()
        rb_psum = tc.tile_pool(name="rbps", bufs=2, space="PSUM")
        rbp = rb_psum.__enter__()

        a2a_in = [nc.dram_tensor(f"a2a_in{b}", [8 * 128, TSB], BF16,
                                 kind="Internal").ap() for b in range(B)]
        a2a_out = [nc.dram_tensor(f"a2a_out{b}", [8 * 128, TSB], BF16,
                                  kind="Internal").ap() for b in range(B)]
        RG8 = [[0, 1, 2, 3, 4, 5, 6, 7]]

        # ---- gpsimd-cheap setup first: memsets (no DMA deps) ----
        warm = st.tile([128, 512], BF16, tag="warm", name="warm")
        nc.gpsimd.memset(warm[:], 0.25)
        ones1 = st.tile([1, 64], BF16, tag="ones1", name="ones1")
        nc.gpsimd.memset(ones1[:], 1.0)
        mask_big = st.tile([128, 896], BF16, tag="mask", name="mask_big")
        nc.gpsimd.memset(mask_big[:], 1.0)
        nc.gpsimd.affine_select(mask_big[:], mask_big[:], pattern=[[1, 896]],
                                compare_op=ALU.is_ge, fill=0.0, base=-384,
                                channel_multiplier=-1)
        v_sb = [[None] * (T // 128) for _ in range(B)]
        for b in range(B):
            for tk in range(T // 128):
                vt = st.tile([128, 2 * 65], BF16, tag=f"v{b}_{tk}",
                             name=f"v_sb{b}_{tk}")
                nc.gpsimd.memset(vt[:], 1.0)
                v_sb[b][tk] = vt

        # ---- warm-up matmuls: run during the initial input DMA window so
        # ---- the HAM clock gate opens and stays open until real work ----
        for wi in range(2):
            acc = ps.tile([128, 512], F32, tag="ps", name=f"wu{wi}")
            for _ in range(18):
                nc.tensor.matmul(acc[:], warm[:, 0:128], warm[:],
                                 start=True, stop=True)

        # ---- input loads ----
        xt_sb = [[None] * CC for _ in range(B)]
        for b in range(B):
            for cc in range(CC):
                xt_sb[b][cc] = big.tile([128, T], BF16, tag="big",
                                        name=f"xt_sb{b}_{cc}")
        wq_sb, wk_sb, wv_sb = [], [], []
        for cc in range(CC):
            for nm, d_, lst in (("k", wk_d, wk_sb), ("q", wq_d, wq_sb),
                                ("v", wv_d, wv_sb)):
                t_ = st.tile([128, 128], BF16, tag=f"w{nm}{cc}", name=f"w{nm}_sb{cc}")
                nc.gpsimd.dma_start(t_[:], d_[cc])
                lst.append(t_)
        # batch-0 x first (t-chunk-major), then batch-1
        for b in range(B):
            for q4 in range(4):
                for cc in range(CC):
                    nc.sync.dma_start(xt_sb[b][cc][:, q4 * 512:(q4 + 1) * 512],
                                      xt_bf[b, cc][:, q4 * 512:(q4 + 1) * 512])
        wp_sb = [[None] * CC for _ in range(8)]
        for s in range(8):
            for cb in range(CC):
                t_ = st.tile([128, 128], BF16, tag=f"wp{s}_{cb}",
                             name=f"wp_sb{s}_{cb}")
                nc.gpsimd.dma_start(t_[:], wp_d[s, cb])
                wp_sb[s][cb] = t_
        bp_sb, b1_sb, b2_sb = [], [], []
        for nm, d_, lst, n in (("bp", bp_d, bp_sb, CC), ("b1", b1_d, b1_sb, FB),
                               ("b2", b2_d, b2_sb, CC)):
            for i in range(n):
                t_ = st.tile([128, 1], F32, tag=f"{nm}{i}", name=f"{nm}_sb{i}")
                nc.gpsimd.dma_start(t_[:], d_[i])
                lst.append(t_)
        xs_sb = []
        for cc in range(CC):
            xs = st.tile([128, TS], F32, tag=f"xs{cc}", name=f"xs{cc}")
            nc.gpsimd.dma_start(xs[:], xts_f[cc])
            xs_sb.append(xs)

        qT = [st.tile([128, T], BF16, tag=f"qT{b}", name=f"qT_sb{b}")
              for b in range(B)]
        kT = [st.tile([128, T], BF16, tag=f"kT{b}", name=f"kT_sb{b}")
              for b in range(B)]
        attnT = [st.tile([128, T], BF16, tag=f"attnT{b}", name=f"attnT_sb{b}")
                 for b in range(B)]
        exq = {}  # (b, qt, hh) -> list of exp tiles

        def qk_half(b, half):
            # per-512-chunk so the first matmul only needs 1MB of x
            for t2 in range(2):
                tt = 2 * half + t2
                for nm, w_sb, dst in (("k", wk_sb, kT), ("q", wq_sb, qT)):
                    acc = psc_pool.tile([128, 512], F32, tag="psc",
                                        name=f"ps_{nm}{b}{tt}")
                    for cc in range(CC):
                        nc.tensor.matmul(
                            acc[:],
                            w_sb[cc][:],
                            xt_sb[b][cc][:, tt * 512:(tt + 1) * 512],
                            start=(cc == 0), stop=(cc == CC - 1),
                        )
                    nc.vector.tensor_copy(
                        dst[b][:, tt * 512:(tt + 1) * 512], acc[:])

        def v_tiles(b, tk0, tk1):
            for tk in range(tk0, tk1):
                vt = v_sb[b][tk]
                acc = ps.tile([128, 128], F32, tag="ps", name=f"ps_v{b}_{tk}")
                for cc in range(CC):
                    nc.tensor.matmul(
                        acc[:],
                        xt_sb[b][cc][:, tk * 128:(tk + 1) * 128],
                        wv_sb[cc][:],
                        start=(cc == 0), stop=(cc == CC - 1),
                    )
                src = acc.rearrange("p (h d) -> p h d", h=2)
                dstv = vt.rearrange("p (h d) -> p h d", h=2, d=65)[:, :, 0:64]
                nc.vector.tensor_copy(dstv, src)

        def scores_qtile(b, qt):
            # scores + exp + mask for both heads of this q-tile. The two hh
            # matmuls sit in disjoint PE row groups (partitions 0-63 vs
            # 64-127), so interleaving them lets the PE overlap.
            nkc = 4 * (qt + 1)
            for b2i in range(nkc // 2):
                sc = [None, None]
                for hh in range(2):
                    sc[hh] = psc_pool.tile([128, 1024], F32, tag="psc",
                                           name=f"psc{b}{hh}{qt}_{b2i}")
                for j in range(2):
                    kc = 2 * b2i + j
                    for hh in range(2):
                        p0 = 64 * hh
                        nc.tensor.matmul(
                            sc[hh][:, j * 512:(j + 1) * 512],
                            kT[b][p0:p0 + 64, kc * 128:(kc + 1) * 128],
                            qT[b][p0:p0 + 64, qt * 512:(qt + 1) * 512],
                            start=True, stop=True,
                        )
                for hh in range(2):
                    ext = expp.tile([128, 1024], BF16, tag="expp",
                                    name=f"ex{b}{hh}{qt}_{b2i}")
                    nc.scalar.activation(ext[:], sc[hh][:], AF.Exp, scale=SCALE)
                    for j in range(2):
                        kc = 2 * b2i + j
                        if kc >= 4 * qt:
                            dd = kc * 128 - qt * 512
                            nc.vector.tensor_mul(
                                ext[:, j * 512:(j + 1) * 512],
                                ext[:, j * 512:(j + 1) * 512],
                                mask_big[:, 384 - dd:896 - dd],
                            )
                    exq.setdefault((b, qt, hh), []).append(ext)

        def weiv_qtile(b, qt):
            nkc = 4 * (qt + 1)
            wv_acc = [ps.tile([65, 512], F32, tag="ps", name=f"pwv{b}{hh}{qt}")
                      for hh in range(2)]
            for b2i in range(nkc // 2):
                for hh in range(2):
                    ext = exq[(b, qt, hh)][b2i]
                    for j in range(2):
                        kc = 2 * b2i + j
                        nc.tensor.matmul(
                            wv_acc[hh][:],
                            v_sb[b][kc][:, hh * 65:hh * 65 + 65],
                            ext[:, j * 512:(j + 1) * 512],
                            start=(kc == 0), stop=(kc == nkc - 1),
                        )
            for hh in range(2):
                p0 = 64 * hh
                den = rcp.tile([1, 512], F32, tag="den", name=f"den{b}{hh}{qt}")
                nc.vector.tensor_copy(den[:], wv_acc[hh][64:65, :])
                rc = rcp.tile([1, 512], F32, tag="rc", name=f"rc{b}{hh}{qt}")
                nc.vector.reciprocal_approx_fast(rc[:], den[:])
                rcb = rcp.tile([1, 512], BF16, tag="rcb", name=f"rcb{b}{hh}{qt}")
                nc.vector.tensor_copy(rcb[:], rc[:])
                rb = rbp.tile([64, 512], F32, tag="rb", name=f"rb{b}{hh}{qt}")
                nc.tensor.matmul(rb[:], ones1[:], rcb[:], start=True, stop=True)
                rbs = rcp.tile([64, 512], BF16, tag="rbs", name=f"rbs{b}{hh}{qt}")
                nc.vector.tensor_copy(rbs[:], rb[:])
                nc.vector.tensor_mul(
                    attnT[b][p0:p0 + 64, qt * 512:(qt + 1) * 512],
                    wv_acc[hh][0:64, :], rbs[:],
                )
            # stage the two dest-shard chunks of this q-tile for the AllToAll
            for j in range(2):
                s = 2 * qt + j
                nc.sync.dma_start(
                    a2a_in[b][s * 128:(s + 1) * 128, :],
                    attnT[b][:, s * TSB:(s + 1) * TSB])

        # ---- attention: batch-major so the b0 exchange overlaps b1 ----
        qk_half(0, 0)
        v_tiles(0, 0, 4)
        scores_qtile(0, 0)
        qk_half(0, 1)
        weiv_qtile(0, 0)
        scores_qtile(0, 1)
        v_tiles(0, 4, 8)
        weiv_qtile(0, 1)
        scores_qtile(0, 2)
        v_tiles(0, 8, 12)
        weiv_qtile(0, 2)
        scores_qtile(0, 3)
        v_tiles(0, 12, 16)
        qk_half(1, 0)
        weiv_qtile(0, 3)

        nc.gpsimd.collective_compute(
            "AllToAll", ALU.bypass, replica_groups=RG8,
            ins=[a2a_in[0]], outs=[a2a_out[0]],
        )
        # receive batch-0 halves of the attn rows (gpsimd queue is idle here)
        rt_sb = [st.tile([128, TS], BF16, tag=f"rt{s}", name=f"rt{s}")
                 for s in range(8)]
        for s in range(8):
            nc.gpsimd.dma_start(rt_sb[s][:, 0:TSB],
                                a2a_out[0][s * 128:(s + 1) * 128, :])

        v_tiles(1, 0, 4)
        scores_qtile(1, 0)
        qk_half(1, 1)
        weiv_qtile(1, 0)
        scores_qtile(1, 1)
        v_tiles(1, 4, 8)
        weiv_qtile(1, 1)
        scores_qtile(1, 2)
        v_tiles(1, 8, 12)
        weiv_qtile(1, 2)
        scores_qtile(1, 3)
        v_tiles(1, 12, 16)

        # w1 loads: emitted after the last xt readers; stream during the
        # b1 tail + exchange window (sync queue).
        w1_sb = [[None, None] for _ in range(CC)]
        for half in range(2):
            for cc in range(CC):
                t_ = big.tile([128, 2048], BF16, tag="big", name=f"w1_sb{cc}_{half}")
                nc.sync.dma_start(t_[:], w1_d[cc][:, half * 2048:(half + 1) * 2048])
                w1_sb[cc][half] = t_

        weiv_qtile(1, 3)

        rb_psum.__exit__(None, None, None)
        attn_psum.__exit__(None, None, None)
        ps_ctx.__exit__(None, None, None)

        # ---- proj: x1 = x + bproj + attn @ Wproj for this core's shard.
        # The batch-0 columns only need the first exchange's data, so they
        # prefill the PE while the batch-1 AllToAll is in flight.
        pj_ctx = tc.tile_pool(name="pj", bufs=8, space="PSUM")
        pj = pj_ctx.__enter__()
        pj_acc = {}

        def proj_half(cb, h0):
            acc = pj_acc[cb]
            for s in range(8):
                nc.tensor.matmul(
                    acc[:, h0 * TSB:(h0 + 1) * TSB],
                    wp_sb[s][cb][:],
                    rt_sb[s][:, h0 * TSB:(h0 + 1) * TSB],
                    start=(s == 0), stop=(s == 7),
                )

        for cb in range(CC):
            pj_acc[cb] = pj.tile([128, TS], F32, tag="pj", name=f"ps_pj{cb}")
            proj_half(cb, 0)

        nc.gpsimd.collective_compute(
            "AllToAll", ALU.bypass, replica_groups=RG8,
            ins=[a2a_in[1]], outs=[a2a_out[1]],
        )
        for s in range(8):
            nc.gpsimd.dma_start(rt_sb[s][:, TSB:TS],
                                a2a_out[1][s * 128:(s + 1) * 128, :])

        x1b = [None] * CC
        for cb in range(CC):
            proj_half(cb, 1)
            xb = st.tile([128, TS], BF16, tag=f"x1b{cb}", name=f"x1b{cb}")
            nc.vector.scalar_tensor_tensor(xb[:], pj_acc[cb][:], bp_sb[cb][:, 0:1],
                                           xs_sb[cb][:], ALU.add, ALU.add)
            x1b[cb] = xb
        pj_ctx.__exit__(None, None, None)
        pf1_ctx = tc.tile_pool(name="pf1", bufs=2, space="PSUM")
        pf1 = pf1_ctx.__enter__()

        # preload the first w2 B-half tiles so FFN2 group B starts instantly
        w2b_sb = {}
        for fc in range(4):
            wt = w2b.tile([128, 512], BF16, tag="w2b", name=f"w2tb{fc}")
            nc.sync.dma_start(wt[:], w2_d[fc][:, 512:1024])
            w2b_sb[fc] = wt

        # ---- FFN1 with FFN2 group A (cb 0-3) interleaved ----
        pf2a_ctx = tc.tile_pool(name="pf2a", bufs=4, space="PSUM")
        pf2a = pf2a_ctx.__enter__()
        accA = [pf2a.tile([128, TS], F32, tag=f"pfa{cb}", bufs=1,
                          name=f"ps_oa{cb}") for cb in range(4)]
        hT = [st.tile([128, TS], BF16, tag=f"hT{fb}", name=f"hT{fb}")
              for fb in range(FB)]
        for fb in range(FB):
            w1h, fo = fb // 16, fb % 16
            acc = pf1.tile([128, TS], F32, tag="pf1", name=f"ps_h{fb}")
            for cc in range(CC):
                nc.tensor.matmul(
                    acc[:],
                    w1_sb[cc][w1h][:, fo * 128:(fo + 1) * 128],
                    x1b[cc][:],
                    start=(cc == 0), stop=(cc == CC - 1))
            nc.scalar.activation(hT[fb][:], acc[:], AF.Relu, bias=b1_sb[fb][:, 0:1])
            wt = w2p.tile([128, 512], BF16, tag="w2s", name=f"w2ta{fb}")
            nc.sync.dma_start(wt[:], w2_d[fb][:, 0:512])
            for cb in range(4):
                nc.tensor.matmul(
                    accA[cb][:], wt[:, cb * 128:(cb + 1) * 128], hT[fb][:],
                    start=(fb == 0), stop=(fb == FB - 1))
        for cb in range(4):
            ot = outp.tile([128, TS], F32, tag="outp", name=f"ot{cb}")
            nc.vector.scalar_tensor_tensor(ot[:], accA[cb][:], b2_sb[cb][:, 0:1],
                                           x1b[cb][:], ALU.add, ALU.add)
            nc.sync.dma_start(out_d[cb], ot[:])
        pf2a_ctx.__exit__(None, None, None)
        pf1_ctx.__exit__(None, None, None)
        pf2_ctx = tc.tile_pool(name="pf2", bufs=4, space="PSUM")
        pf2 = pf2_ctx.__enter__()

        # ---- FFN2 group B ----
        accB = [pf2.tile([128, TS], F32, tag=f"pfb{cb}", bufs=1,
                         name=f"ps_ob{cb}") for cb in range(4)]
        for fc in range(FB):
            if fc in w2b_sb:
                wt = w2b_sb[fc]
            else:
                wt = w2b.tile([128, 512], BF16, tag="w2b", name=f"w2tb{fc}")
                nc.gpsimd.dma_start(wt[:], w2_d[fc][:, 512:1024])
            for cb in range(4):
                nc.tensor.matmul(
                    accB[cb][:], wt[:, cb * 128:(cb + 1) * 128], hT[fc][:],
                    start=(fc == 0), stop=(fc == FB - 1))
        for cb4 in range(4):
            cb = cb4 + 4
            ot = outp.tile([128, TS], F32, tag="outp", name=f"ot{cb}")
            nc.vector.scalar_tensor_tensor(ot[:], accB[cb4][:], b2_sb[cb][:, 0:1],
                                           x1b[cb][:], ALU.add, ALU.add)
            nc.sync.dma_start(out_d[cb], ot[:])

        pf2_ctx.__exit__(None, None, None)


_CACHED = None


def _get_compiled():
    global _CACHED
    if _CACHED is None:
        nc = bacc.Bacc("TRN2", target_bir_lowering=False, debug=False,
                       num_devices=N_CORES)
        build_program(nc)
        nc.compile()
        _CACHED = nc
    return _CACHED


def _prep_inputs(x, Wq, Wk, Wv, Wproj, bproj, W1, b1, W2, b2):
    bf = ml_dtypes.bfloat16
    W1t = np.ascontiguousarray(W1.astype(bf).reshape(CC, 128, F))
    W2t = np.ascontiguousarray(W2.astype(bf).reshape(FB, 128, C))
    b1r = np.ascontiguousarray(b1.astype(np.float32).reshape(FB, 128, 1))
    b2r = np.ascontiguousarray(b2.astype(np.float32).reshape(CC, 128, 1))
    bpr = np.ascontiguousarray(bproj.astype(np.float32).reshape(CC, 128, 1))
    # full Wproj on every core, rows grouped by source core (head pair)
    wp_s = np.ascontiguousarray(
        Wproj.astype(bf).reshape(8, 128, CC, 128).transpose(0, 2, 1, 3))
    # x transposed, both batches, shared by all cores
    xT = [np.ascontiguousarray(x[b].T.astype(np.float32)) for b in range(B)]
    xT_bf = np.ascontiguousarray(
        np.stack([xT[b].astype(bf).reshape(CC, 128, T) for b in range(B)]))

    in_maps = []
    for c in range(N_CORES):
        cols = slice(128 * c, 128 * (c + 1))
        wq_s = np.ascontiguousarray(Wq[:, cols].astype(bf).reshape(CC, 128, 128))
        wk_s = np.ascontiguousarray(Wk[:, cols].astype(bf).reshape(CC, 128, 128))
        wv_s = np.ascontiguousarray(Wv[:, cols].astype(bf).reshape(CC, 128, 128))
        tok = slice(TSB * c, TSB * (c + 1))
        xts = np.ascontiguousarray(
            np.concatenate([xT[0][:, tok], xT[1][:, tok]], axis=1)
            .reshape(CC, 128, TS))
        in_maps.append({
            "xt_bf": xT_bf, "xts_f": xts,
            "wq": wq_s, "wk": wk_s, "wv": wv_s, "wp": wp_s,
            "w1": W1t, "w2": W2t, "bp": bpr, "b1": b1r, "b2": b2r,
        })
    return in_maps


def kernel(x, Wq, Wk, Wv, Wproj, bproj, W1, b1, W2, b2, _trace=False):
    nc = _get_compiled()
    in_maps = _prep_inputs(np.asarray(x), np.asarray(Wq), np.asarray(Wk),
                           np.asarray(Wv), np.asarray(Wproj), np.asarray(bproj),
                           np.asarray(W1), np.asarray(b1), np.asarray(W2),
                           np.asarray(b2))
    res = run_bass_kernel_spmd(nc, in_maps, list(range(N_CORES)), trace=_trace)
    out = np.empty((B, T, C), dtype=np.float32)
    for c in range(N_CORES):
        shard = res.results[c]["outT"].reshape(C, TS)
        for b in range(B):
            out[b, TSB * c: TSB * (c + 1), :] = shard[:, TSB * b:TSB * (b + 1)].T
    if _trace:
        kernel.last_exec_time_ns = res.exec_time_ns
    return out



# revision 15
# speedup vs baseline: 1.0901x; 1.0901x over previous
"""Fused transformer block (attention + MLP) on 8 trn2 NeuronCores.

Sharding (8-way, batch-symmetric): every core computes attention for ONE
head-pair (heads 2c, 2c+1) of BOTH batches, and owns a 256-token shard of
each batch (tokens [256c, 256c+256)). The two shard halves are concatenated
along the free axis into one 512-column tile set, so projection + FFN code
is identical to a single 512-token shard.

Schedule: the attention inner loop is software-pipelined at k-chunk-pair
granularity: scores(b2i) -> exp(b2i) on the scalar engine -> weiv(b2i)
trailing one chunk behind, with qk / v-tile units for the NEXT q-tile (or
next batch) interleaved as tensor-engine filler so the PE never idles long
enough for the HAM clock gate to re-throttle. PSUM: scores ring 2x2 banks,
weiv accumulators 2x1, qk/v/rb scratch ring 2x1 = 8 banks.

All weight/bias loads are batched host-side (wp 1 DMA, biases 3, xs 1) and
issued from the sync queue (hardware DGE) instead of gpsimd (software DGE,
~1us per descriptor). gpsimd keeps only memsets, collectives and the a2a
receive DMAs. a2a staging DMAs ride the vector queue right behind the
normalization muls that produce them.

FFN: FFN2 is split 6+2 (not 4+4): 6 column blocks accumulate interleaved
inside the FFN1 loop (PSUM: 2 FFN1 ring + 6 accumulators), the last 2 run
in a short tail that reuses the FFN1 ring slots (no pool-close bubble) and
whose w2 slices are fully preloaded during batch-1 attention.
"""

import sys

for _p in ("/opt/trn_rl_repo",):
    if _p not in sys.path:
        sys.path.append(_p)

import numpy as np
import ml_dtypes

import concourse.bass as bass
import concourse.tile as tile
from concourse import bacc, mybir
from concourse.bass_utils import run_bass_kernel_spmd

BF16 = mybir.dt.bfloat16
F32 = mybir.dt.float32
AF = mybir.ActivationFunctionType
ALU = mybir.AluOpType

N_CORES = 8
B, T, C = 2, 2048, 1024
H, HS = 16, 64
F = 4 * C
TS = 512          # per-core fused shard width (256 tokens x 2 batches)
TSB = 256         # per-batch shard width
CC = C // 128
FB = F // 128
SCALE = float(C) ** -0.5
NA = 6            # FFN2 group-A column blocks (interleaved in FFN1)


def build_program(nc: bass.Bass):
    xt_bf = nc.dram_tensor("xt_bf", [B, CC, 128, T], BF16,
                           kind="ExternalInput").ap()
    xs_d = nc.dram_tensor("xs", [128, CC * TS], F32, kind="ExternalInput").ap()
    wq_d = nc.dram_tensor("wq", [128, C], BF16, kind="ExternalInput").ap()
    wk_d = nc.dram_tensor("wk", [128, C], BF16, kind="ExternalInput").ap()
    wv_d = nc.dram_tensor("wv", [128, C], BF16, kind="ExternalInput").ap()
    wp_d = nc.dram_tensor("wp", [128, 8 * CC * 128], BF16,
                          kind="ExternalInput").ap()
    w1_d = nc.dram_tensor("w1", [CC, 128, F], BF16, kind="ExternalInput").ap()
    w2_d = nc.dram_tensor("w2", [FB, 128, C], BF16, kind="ExternalInput").ap()
    bp_d = nc.dram_tensor("bp", [128, CC], F32, kind="ExternalInput").ap()
    b1_d = nc.dram_tensor("b1", [128, FB], F32, kind="ExternalInput").ap()
    b2_d = nc.dram_tensor("b2", [128, CC], F32, kind="ExternalInput").ap()
    out_d = nc.dram_tensor("outT", [CC, 128, TS], F32, kind="ExternalOutput").ap()

    with tile.TileContext(nc) as tc:
        _emit(nc, tc, xt_bf, xs_d, wq_d, wk_d, wv_d, wp_d, w1_d, w2_d,
              bp_d, b1_d, b2_d, out_d)


def _emit(nc, tc, xt_bf, xs_d, wq_d, wk_d, wv_d, wp_d, w1_d, w2_d,
          bp_d, b1_d, b2_d, out_d):
    from contextlib import ExitStack

    ctx = ExitStack()
    with ctx:
        st = ctx.enter_context(tc.tile_pool(name="static", bufs=1))
        big = ctx.enter_context(tc.tile_pool(name="big", bufs=16))
        expp = ctx.enter_context(tc.tile_pool(name="expp", bufs=4))
        w2p = ctx.enter_context(tc.tile_pool(name="w2s", bufs=4))
        w2bp = ctx.enter_context(tc.tile_pool(name="w2bp", bufs=8))
        outp = ctx.enter_context(tc.tile_pool(name="outp", bufs=2))
        rcp = ctx.enter_context(tc.tile_pool(name="rcp", bufs=1))
        stgp = ctx.enter_context(tc.tile_pool(name="stgp", bufs=2))

        # attention PSUM pools (closed before proj)
        ps_ctx = tc.tile_pool(name="ps", bufs=2, space="PSUM")
        ps = ps_ctx.__enter__()
        sc_ctx = tc.tile_pool(name="scps", bufs=2, space="PSUM")
        scp = sc_ctx.__enter__()
        wv_ctx = tc.tile_pool(name="wvps", bufs=2, space="PSUM")
        wvp = wv_ctx.__enter__()

        a2a_in = [nc.dram_tensor(f"a2a_in{b}", [8 * 128, TSB], BF16,
                                 kind="Internal").ap() for b in range(B)]
        a2a_out = [nc.dram_tensor(f"a2a_out{b}", [8 * 128, TSB], BF16,
                                  kind="Internal").ap() for b in range(B)]
        RG8 = [[0, 1, 2, 3, 4, 5, 6, 7]]

        # ---- gpsimd-cheap setup first: memsets (no DMA deps) ----
        ones1 = st.tile([1, 64], BF16, tag="ones1", name="ones1")
        nc.gpsimd.memset(ones1[:], 1.0)
        mask_big = st.tile([128, 896], BF16, tag="mask", name="mask_big")
        nc.gpsimd.memset(mask_big[:], 1.0)
        nc.gpsimd.affine_select(mask_big[:], mask_big[:], pattern=[[1, 896]],
                                compare_op=ALU.is_ge, fill=0.0, base=-384,
                                channel_multiplier=-1)
        v_sb = [[None] * (T // 128) for _ in range(B)]
        for b in range(B):
            for tk in range(T // 128):
                vt = st.tile([128, 2 * 65], BF16, tag=f"v{b}_{tk}",
                             name=f"v_sb{b}_{tk}")
                nc.gpsimd.memset(vt[:], 1.0)
                v_sb[b][tk] = vt

        # ---- warm-up matmuls: run during the initial input DMA window
        # ---- (mask_big doubles as the warm-up operand) ----
        for wi in range(2):
            acc = ps.tile([128, 512], F32, tag="ps", name=f"wu{wi}")
            for _ in range(18):
                nc.tensor.matmul(acc[:], mask_big[:, 0:128],
                                 mask_big[:, 128:640], start=True, stop=True)

        # ---- input loads: all on the sync queue (hardware DGE) ----
        xt_sb = [[None] * CC for _ in range(B)]
        for b in range(B):
            for cc in range(CC):
                xt_sb[b][cc] = big.tile([128, T], BF16, tag="big",
                                        name=f"xt_sb{b}_{cc}")
        # batch-0 x first, t-chunk-major so qt=0 work can start early
        for q4 in range(4):
            for cc in range(CC):
                nc.sync.dma_start(xt_sb[0][cc][:, q4 * 512:(q4 + 1) * 512],
                                  xt_bf[0, cc][:, q4 * 512:(q4 + 1) * 512])
        wqkv_t = {}
        for nm, d_ in (("k", wk_d), ("q", wq_d), ("v", wv_d)):
            t_ = st.tile([128, C], BF16, tag=f"w{nm}", name=f"w{nm}_t")
            nc.sync.dma_start(t_[:], d_)
            wqkv_t[nm] = t_
        for cc in range(CC):
            nc.sync.dma_start(xt_sb[1][cc][:], xt_bf[1, cc][:])
        wp_t = st.tile([128, 8 * CC * 128], BF16, tag="wp", name="wp_t")
        nc.sync.dma_start(wp_t[:], wp_d)
        bp_t = st.tile([128, CC], F32, tag="bp", name="bp_t")
        nc.sync.dma_start(bp_t[:], bp_d)
        b1_t = st.tile([128, FB], F32, tag="b1", name="b1_t")
        nc.sync.dma_start(b1_t[:], b1_d)
        b2_t = st.tile([128, CC], F32, tag="b2", name="b2_t")
        nc.sync.dma_start(b2_t[:], b2_d)

        qT = [st.tile([128, T], BF16, tag=f"qT{b}", name=f"qT_sb{b}")
              for b in range(B)]
        kT = [st.tile([128, T], BF16, tag=f"kT{b}", name=f"kT_sb{b}")
              for b in range(B)]
        exq = {}  # (b, qt, hh, b2i) -> exp tile

        # ---------- unit emitters ----------
        def qk_unit(b, nm, tt):
            # one 512-token slice of qT/kT for batch b
            dst = kT[b] if nm == "k" else qT[b]
            acc = ps.tile([128, 512], F32, tag="ps", name=f"pqk_{nm}{b}{tt}")
            w_t = wqkv_t[nm]
            for cc in range(CC):
                nc.tensor.matmul(
                    acc[:],
                    w_t[:, cc * 128:(cc + 1) * 128],
                    xt_sb[b][cc][:, tt * 512:(tt + 1) * 512],
                    start=(cc == 0), stop=(cc == CC - 1),
                )
            nc.vector.tensor_copy(dst[:, tt * 512:(tt + 1) * 512], acc[:])

        def v_unit(b, tk):
            vt = v_sb[b][tk]
            acc = ps.tile([128, 128], F32, tag="ps", name=f"ps_v{b}_{tk}")
            for cc in range(CC):
                nc.tensor.matmul(
                    acc[:],
                    xt_sb[b][cc][:, tk * 128:(tk + 1) * 128],
                    wqkv_t["v"][:, cc * 128:(cc + 1) * 128],
                    start=(cc == 0), stop=(cc == CC - 1),
                )
            src = acc.rearrange("p (h d) -> p h d", h=2)
            dstv = vt.rearrange("p (h d) -> p h d", h=2, d=65)[:, :, 0:64]
            nc.vector.tensor_copy(dstv, src)

        def s_unit(b, qt, b2i):
            # scores + exp (+ causal mask) for k-chunks 2*b2i, 2*b2i+1.
            # The two hh matmuls sit in disjoint PE row groups (partitions
            # 0-63 vs 64-127) so they run concurrently.
            sc = [None, None]
            for hh in range(2):
                sc[hh] = scp.tile([128, 1024], F32, tag="sc",
                                  name=f"psc{b}{hh}{qt}_{b2i}")
            for j in range(2):
                kc = 2 * b2i + j
                for hh in range(2):
                    p0 = 64 * hh
                    nc.tensor.matmul(
                        sc[hh][:, j * 512:(j + 1) * 512],
                        kT[b][p0:p0 + 64, kc * 128:(kc + 1) * 128],
                        qT[b][p0:p0 + 64, qt * 512:(qt + 1) * 512],
                        start=True, stop=True,
                    )
            for hh in range(2):
                ext = expp.tile([128, 1024], BF16, tag="expp",
                                name=f"ex{b}{hh}{qt}_{b2i}")
                nc.scalar.activation(ext[:], sc[hh][:], AF.Exp, scale=SCALE)
                for j in range(2):
                    kc = 2 * b2i + j
                    if kc >= 4 * qt:
                        dd = kc * 128 - qt * 512
                        nc.vector.tensor_mul(
                            ext[:, j * 512:(j + 1) * 512],
                            ext[:, j * 512:(j + 1) * 512],
                            mask_big[:, 384 - dd:896 - dd],
                        )
                exq[(b, qt, hh, b2i)] = ext

        def w_unit(b, qt, b2i, wv_acc):
            nkc = 4 * (qt + 1)
            for j in range(2):
                kc = 2 * b2i + j
                for hh in range(2):
                    ext = exq[(b, qt, hh, b2i)]
                    nc.tensor.matmul(
                        wv_acc[hh][:],
                        v_sb[b][kc][:, hh * 65:hh * 65 + 65],
                        ext[:, j * 512:(j + 1) * 512],
                        start=(kc == 0), stop=(kc == nkc - 1),
                    )

        def f_unit(b, qt, wv_acc):
            # normalize by the softmax denominator (the ones-column of V)
            stg = stgp.tile([128, 512], BF16, tag="stg", name=f"stg{b}{qt}")
            for hh in range(2):
                p0 = 64 * hh
                den = rcp.tile([1, 512], F32, tag="den", name=f"den{b}{hh}{qt}")
                nc.vector.tensor_copy(den[:], wv_acc[hh][64:65, :])
                rc = rcp.tile([1, 512], F32, tag="rc", name=f"rc{b}{hh}{qt}")
                nc.vector.reciprocal_approx_fast(rc[:], den[:])
                rcb = rcp.tile([1, 512], BF16, tag="rcb", name=f"rcb{b}{hh}{qt}")
                nc.vector.tensor_copy(rcb[:], rc[:])
                rb = ps.tile([64, 512], F32, tag="ps", name=f"rb{b}{hh}{qt}")
                nc.tensor.matmul(rb[:], ones1[:], rcb[:], start=True, stop=True)
                rbs = rcp.tile([64, 512], BF16, tag="rbs", name=f"rbs{b}{hh}{qt}")
                nc.vector.tensor_copy(rbs[:], rb[:])
                nc.vector.tensor_mul(stg[p0:p0 + 64, :], wv_acc[hh][0:64, :],
                                     rbs[:])
            # stage the two dest-shard chunks for the AllToAll (gpsimd queue
            # so they don't sit behind weight loads on the sync DMA queue)
            for j in range(2):
                s = 2 * qt + j
                nc.gpsimd.dma_start(
                    a2a_in[b][s * 128:(s + 1) * 128, :],
                    stg[:, j * TSB:(j + 1) * TSB])

        # ---------- attention schedule ----------
        # fillers(b, qt) = prerequisite units of the NEXT qt block, emitted
        # as PE filler between the scalar-gated s/w steps of this block.
        fillers = {
            (0, 0): [("qk", 0, "k", 1), ("qk", 0, "q", 1)] +
                    [("v", 0, tk) for tk in range(4, 8)],
            (0, 1): [("qk", 0, "k", 2), ("qk", 0, "q", 2)] +
                    [("v", 0, tk) for tk in range(8, 12)],
            (0, 2): [("qk", 0, "k", 3), ("qk", 0, "q", 3)] +
                    [("v", 0, tk) for tk in range(12, 16)],
            (0, 3): [("qk", 1, "k", 0), ("qk", 1, "q", 0)] +
                    [("v", 1, tk) for tk in range(0, 4)],
            (1, 0): [("qk", 1, "k", 1), ("qk", 1, "q", 1)] +
                    [("v", 1, tk) for tk in range(4, 8)],
            (1, 1): [("qk", 1, "k", 2), ("qk", 1, "q", 2)] +
                    [("v", 1, tk) for tk in range(8, 12)],
            (1, 2): [("qk", 1, "q", 3), ("v", 1, 12), ("v", 1, 13)],
            (1, 3): [("qk", 1, "k", 3), ("v", 1, 14), ("v", 1, 15)],
        }

        def emit_filler(u):
            if u[0] == "qk":
                qk_unit(u[1], u[2], u[3])
            else:
                v_unit(u[1], u[2])

        def qt_block(b, qt, post=None):
            nb2i = 2 * (qt + 1)
            fl = list(fillers[(b, qt)])
            # for (1,3) the fillers are this block's OWN late prerequisites:
            # kT(1,3) is only needed from b2i=6, v(1,14/15) from b2i=7.
            own_late = (b, qt) == (1, 3)
            wv_acc = [wvp.tile([65, 512], F32, tag="wv",
                               name=f"pwv{b}{hh}{qt}")
                      for hh in range(2)]
            for i in range(nb2i):
                if own_late:
                    if i == 2 and fl:
                        emit_filler(fl.pop(0))       # kT(1,3)
                    if i == 4 and len(fl) == 2:
                        emit_filler(fl.pop(0))       # v(1,14)
                        emit_filler(fl.pop(0))       # v(1,15)
                s_unit(b, qt, i)
                if i >= 1:
                    w_unit(b, qt, i - 1, wv_acc)
                if not own_late:
                    if fl:
                        emit_filler(fl.pop(0))
                    if fl and len(fl) > nb2i - 1 - i:
                        emit_filler(fl.pop(0))
            while fl:
                emit_filler(fl.pop(0))
            w_unit(b, qt, nb2i - 1, wv_acc)
            f_unit(b, qt, wv_acc)
            if post is not None:
                post()

        # batch 0 prerequisites
        qk_unit(0, "k", 0)
        qk_unit(0, "q", 0)
        for tk in range(4):
            v_unit(0, tk)

        for qt in range(4):
            qt_block(0, qt)

        nc.gpsimd.collective_compute(
            "AllToAll", ALU.bypass, replica_groups=RG8,
            ins=[a2a_in[0]], outs=[a2a_out[0]],
        )
        rt_sb = [st.tile([128, TS], BF16, tag=f"rt{s}", name=f"rt{s}")
                 for s in range(8)]

        def emit_rcv0():
            # batch-0 receive DMAs; emitted after stg(1,1) so their wait on
            # the first AllToAll never blocks batch-1 staging on gpsimd
            for s in range(8):
                nc.gpsimd.dma_start(rt_sb[s][:, 0:TSB],
                                    a2a_out[0][s * 128:(s + 1) * 128, :])

        def emit_w1_loads():
            # stream during batch-1 attention on the sync queue
            for half in range(2):
                for cc in range(CC):
                    t_ = big.tile([128, 2048], BF16, tag="big",
                                  name=f"w1_sb{cc}_{half}")
                    nc.sync.dma_start(
                        t_[:], w1_d[cc][:, half * 2048:(half + 1) * 2048])
                    w1_sb[cc][half] = t_

        w1_sb = [[None, None] for _ in range(CC)]

        def post_11():
            emit_rcv0()
            emit_w1_loads()

        qt_block(1, 0)
        qt_block(1, 1, post=post_11)
        qt_block(1, 2)
        qt_block(1, 3)

        wv_ctx.__exit__(None, None, None)
        sc_ctx.__exit__(None, None, None)
        ps_ctx.__exit__(None, None, None)

        # ---- proj: x1 = x + bproj + attn @ Wproj for this core's shard.
        # Batch-0 columns only need the first exchange, so they prefill the
        # PE while the batch-1 AllToAll is in flight.
        pj_ctx = tc.tile_pool(name="pj", bufs=8, space="PSUM")
        pj = pj_ctx.__enter__()
        pj_acc = {}

        def proj_half(cb, h0):
            acc = pj_acc[cb]
            for s in range(8):
                nc.tensor.matmul(
                    acc[:, h0 * TSB:(h0 + 1) * TSB],
                    wp_t[:, (s * CC + cb) * 128:(s * CC + cb + 1) * 128],
                    rt_sb[s][:, h0 * TSB:(h0 + 1) * TSB],
                    start=(s == 0), stop=(s == 7),
                )

        for cb in range(CC):
            pj_acc[cb] = pj.tile([128, TS], F32, tag="pj", name=f"ps_pj{cb}")
            proj_half(cb, 0)

        nc.gpsimd.collective_compute(
            "AllToAll", ALU.bypass, replica_groups=RG8,
            ins=[a2a_in[1]], outs=[a2a_out[1]],
        )
        for s in range(8):
            nc.gpsimd.dma_start(rt_sb[s][:, TSB:TS],
                                a2a_out[1][s * 128:(s + 1) * 128, :])

        # xs (fp32 residual), loaded late so it never crowds the weight loads
        xs_sb = []
        for xi in range(4):
            xst = big.tile([128, 1024], F32, tag="xsb", bufs=4, name=f"xs{xi}")
            nc.sync.dma_start(xst[:], xs_d[:, xi * 1024:(xi + 1) * 1024])
            xs_sb.append(xst)

        def xs_slice(cb):
            return xs_sb[cb // 2][:, (cb % 2) * 512:(cb % 2) * 512 + 512]

        x1b = [None] * CC
        for cb in range(CC):
            proj_half(cb, 1)
            xb = st.tile([128, TS], BF16, tag=f"x1b{cb}", name=f"x1b{cb}")
            nc.vector.scalar_tensor_tensor(xb[:], pj_acc[cb][:],
                                           bp_t[:, cb:cb + 1],
                                           xs_slice(cb), ALU.add, ALU.add)
            x1b[cb] = xb
        pj_ctx.__exit__(None, None, None)

        # ---- FFN: FFN1 with FFN2 group A (cb 0..NA-1) interleaved ----
        pf1_ctx = tc.tile_pool(name="pf1", bufs=2, space="PSUM")
        pf1 = pf1_ctx.__enter__()
        pfa_ctx = tc.tile_pool(name="pfa", bufs=NA, space="PSUM")
        pfa = pfa_ctx.__enter__()
        accA = [pfa.tile([128, TS], F32, tag=f"pfa{cb}", bufs=1,
                         name=f"ps_oa{cb}") for cb in range(NA)]
        hT = [st.tile([128, TS], BF16, tag=f"hT{fb}", name=f"hT{fb}")
              for fb in range(FB)]
        w2b_sb = []
        for fb in range(FB):
            w1h, fo = fb // 16, fb % 16
            acc = pf1.tile([128, TS], F32, tag="pf1", name=f"ps_h{fb}")
            for cc in range(CC):
                nc.tensor.matmul(
                    acc[:],
                    w1_sb[cc][w1h][:, fo * 128:(fo + 1) * 128],
                    x1b[cc][:],
                    start=(cc == 0), stop=(cc == CC - 1))
            nc.scalar.activation(hT[fb][:], acc[:], AF.Relu,
                                 bias=b1_t[:, fb:fb + 1])
            wt = w2p.tile([128, NA * 128], BF16, tag="w2s", name=f"w2ta{fb}")
            nc.sync.dma_start(wt[:], w2_d[fb][:, 0:NA * 128])
            wtb = w2bp.tile([128, (CC - NA) * 128], BF16, tag="w2b",
                            name=f"w2tb{fb}")
            nc.sync.dma_start(wtb[:], w2_d[fb][:, NA * 128:C])
            w2b_sb.append(wtb)
            for cb in range(NA):
                nc.tensor.matmul(
                    accA[cb][:], wt[:, cb * 128:(cb + 1) * 128], hT[fb][:],
                    start=(fb == 0), stop=(fb == FB - 1))

        # ---- FFN2 group B (cb NA..7): reuses the pf1 ring slots ----
        accB = [pf1.tile([128, TS], F32, tag="pf1", name=f"ps_ob{cb}")
                for cb in range(CC - NA)]
        first = True
        for fc in range(FB):
            wt = w2b_sb[fc]
            for cb in range(CC - NA):
                nc.tensor.matmul(
                    accB[cb][:], wt[:, cb * 128:(cb + 1) * 128], hT[fc][:],
                    start=(fc == 0), stop=(fc == FB - 1))
            if first:
                first = False
                # group-A outputs drain while B accumulates
                for cb in range(NA):
                    ot = outp.tile([128, TS], F32, tag="outp", name=f"ot{cb}")
                    nc.vector.scalar_tensor_tensor(
                        ot[:], accA[cb][:], b2_t[:, cb:cb + 1],
                        x1b[cb][:], ALU.add, ALU.add)
                    nc.sync.dma_start(out_d[cb], ot[:])
        for cb4 in range(CC - NA):
            cb = cb4 + NA
            ot = outp.tile([128, TS], F32, tag="outp", name=f"ot{cb}")
            nc.vector.scalar_tensor_tensor(ot[:], accB[cb4][:],
                                           b2_t[:, cb:cb + 1],
                                           x1b[cb][:], ALU.add, ALU.add)
            nc.sync.dma_start(out_d[cb], ot[:])

        pfa_ctx.__exit__(None, None, None)
        pf1_ctx.__exit__(None, None, None)


_CACHED = None


def _get_compiled():
    global _CACHED
    if _CACHED is None:
        nc = bacc.Bacc("TRN2", target_bir_lowering=False, debug=False,
                       num_devices=N_CORES)
        build_program(nc)
        nc.compile()
        _CACHED = nc
    return _CACHED


def _prep_inputs(x, Wq, Wk, Wv, Wproj, bproj, W1, b1, W2, b2):
    bf = ml_dtypes.bfloat16
    W1t = np.ascontiguousarray(W1.astype(bf).reshape(CC, 128, F))
    W2t = np.ascontiguousarray(W2.astype(bf).reshape(FB, 128, C))
    b1r = np.ascontiguousarray(b1.astype(np.float32).reshape(FB, 128).T)
    b2r = np.ascontiguousarray(b2.astype(np.float32).reshape(CC, 128).T)
    bpr = np.ascontiguousarray(bproj.astype(np.float32).reshape(CC, 128).T)
    # full Wproj on every core: wp_flat[p, (s*CC+cb)*128+k] = Wproj[128s+p, 128cb+k]
    wp_flat = np.ascontiguousarray(
        Wproj.astype(bf).reshape(8, 128, CC, 128).transpose(1, 0, 2, 3)
        .reshape(128, 8 * CC * 128))
    # x transposed, both batches, shared by all cores
    xT = [np.ascontiguousarray(x[b].T.astype(np.float32)) for b in range(B)]
    xT_bf = np.ascontiguousarray(
        np.stack([xT[b].astype(bf).reshape(CC, 128, T) for b in range(B)]))

    in_maps = []
    for c in range(N_CORES):
        cols = slice(128 * c, 128 * (c + 1))
        # wq_t[p, cc*128+k] = Wq[128cc+p, core_cols[k]]
        wq_s = np.ascontiguousarray(
            Wq[:, cols].astype(bf).reshape(CC, 128, 128)
            .transpose(1, 0, 2).reshape(128, C))
        wk_s = np.ascontiguousarray(
            Wk[:, cols].astype(bf).reshape(CC, 128, 128)
            .transpose(1, 0, 2).reshape(128, C))
        wv_s = np.ascontiguousarray(
            Wv[:, cols].astype(bf).reshape(CC, 128, 128)
            .transpose(1, 0, 2).reshape(128, C))
        tok = slice(TSB * c, TSB * (c + 1))
        # xs_t[p, cb*512+j] = fused-shard residual, fp32
        xts = np.ascontiguousarray(
            np.concatenate([xT[0][:, tok], xT[1][:, tok]], axis=1)
            .reshape(CC, 128, TS).transpose(1, 0, 2).reshape(128, CC * TS))
        in_maps.append({
            "xt_bf": xT_bf, "xs": xts,
            "wq": wq_s, "wk": wk_s, "wv": wv_s, "wp": wp_flat,
            "w1": W1t, "w2": W2t, "bp": bpr, "b1": b1r, "b2": b2r,
        })
    return in_maps


def kernel(x, Wq, Wk, Wv, Wproj, bproj, W1, b1, W2, b2, _trace=False):
    nc = _get_compiled()
    in_maps = _prep_inputs(np.asarray(x), np.asarray(Wq), np.asarray(Wk),
                           np.asarray(Wv), np.asarray(Wproj), np.asarray(bproj),
                           np.asarray(W1), np.asarray(b1), np.asarray(W2),
                           np.asarray(b2))
    res = run_bass_kernel_spmd(nc, in_maps, list(range(N_CORES)), trace=_trace)
    out = np.empty((B, T, C), dtype=np.float32)
    for c in range(N_CORES):
        shard = res.results[c]["outT"].reshape(C, TS)
        for b in range(B):
            out[b, TSB * c: TSB * (c + 1), :] = shard[:, TSB * b:TSB * (b + 1)].T
    if _trace:
        kernel.last_exec_time_ns = res.exec_time_ns
    return out


# revision 22
# speedup vs baseline: 1.2021x; 1.1026x over previous
"""Fused transformer block (attention + MLP) on 8 trn2 NeuronCores.

Sharding (8-way, batch-symmetric): every core computes attention for ONE
head-pair (heads 2c, 2c+1) of BOTH batches, and owns a 256-token shard of
each batch (tokens [256c, 256c+256)). The two shard halves are concatenated
along the free axis into one 512-column tile set, so projection + FFN code
is identical to a single 512-token shard.

Schedule: the attention inner loop is software-pipelined at k-chunk-pair
granularity: scores(b2i) -> exp(b2i) on the scalar engine -> weiv(b2i)
trailing one chunk behind, with qk / v-tile units for the NEXT q-tile (or
next batch) interleaved as tensor-engine filler so the PE never idles long
enough for the HAM clock gate to re-throttle. PSUM: scores ring 2x2 banks,
weiv accumulators 2x1, qk/v/rb scratch ring 2x1 = 8 banks.

All weight/bias loads are batched host-side (wp 1 DMA, biases 3, xs 1) and
issued from the sync queue (hardware DGE) instead of gpsimd (software DGE,
~1us per descriptor). gpsimd keeps only memsets, collectives and the a2a
receive DMAs. a2a staging DMAs ride the vector queue right behind the
normalization muls that produce them.

FFN: FFN2 is split 6+2 (not 4+4): 6 column blocks accumulate interleaved
inside the FFN1 loop (PSUM: 2 FFN1 ring + 6 accumulators), the last 2 run
in a short tail that reuses the FFN1 ring slots (no pool-close bubble) and
whose w2 slices are fully preloaded during batch-1 attention.
"""

import sys

for _p in ("/opt/trn_rl_repo",):
    if _p not in sys.path:
        sys.path.append(_p)

import numpy as np
import ml_dtypes

import concourse.bass as bass
import concourse.tile as tile
from concourse import bacc, mybir
from concourse.bass_utils import run_bass_kernel_spmd

BF16 = mybir.dt.bfloat16
F32 = mybir.dt.float32
AF = mybir.ActivationFunctionType
ALU = mybir.AluOpType

N_CORES = 8
B, T, C = 2, 2048, 1024
H, HS = 16, 64
F = 4 * C
TS = 512          # per-core fused shard width (256 tokens x 2 batches)
TSB = 256         # per-batch shard width
CC = C // 128
FB = F // 128
SCALE = float(C) ** -0.5
NA = 6            # FFN2 group-A column blocks (interleaved in FFN1)


def build_program(nc: bass.Bass):
    xt_bf = nc.dram_tensor("xt_bf", [B, CC, 128, T], BF16,
                           kind="ExternalInput").ap()
    xs_d = nc.dram_tensor("xs", [128, CC * TS], F32, kind="ExternalInput").ap()
    wq_d = nc.dram_tensor("wq", [128, C], BF16, kind="ExternalInput").ap()
    wk_d = nc.dram_tensor("wk", [128, C], BF16, kind="ExternalInput").ap()
    wv_d = nc.dram_tensor("wv", [128, C], BF16, kind="ExternalInput").ap()
    wp_d = nc.dram_tensor("wp", [128, 8 * CC * 128], BF16,
                          kind="ExternalInput").ap()
    w1_d = nc.dram_tensor("w1", [CC, 128, F], BF16, kind="ExternalInput").ap()
    w2_d = nc.dram_tensor("w2", [FB, 128, C], BF16, kind="ExternalInput").ap()
    bp_d = nc.dram_tensor("bp", [128, CC], F32, kind="ExternalInput").ap()
    b1_d = nc.dram_tensor("b1", [128, FB], F32, kind="ExternalInput").ap()
    b2_d = nc.dram_tensor("b2", [128, CC], F32, kind="ExternalInput").ap()
    out_d = nc.dram_tensor("outT", [CC, 128, TS], F32, kind="ExternalOutput").ap()

    with tile.TileContext(nc) as tc:
        _emit(nc, tc, xt_bf, xs_d, wq_d, wk_d, wv_d, wp_d, w1_d, w2_d,
              bp_d, b1_d, b2_d, out_d)


def _emit(nc, tc, xt_bf, xs_d, wq_d, wk_d, wv_d, wp_d, w1_d, w2_d,
          bp_d, b1_d, b2_d, out_d):
    from contextlib import ExitStack

    ctx = ExitStack()
    with ctx:
        st = ctx.enter_context(tc.tile_pool(name="static", bufs=1))
        big = ctx.enter_context(tc.tile_pool(name="big", bufs=16))
        expp = ctx.enter_context(tc.tile_pool(name="expp", bufs=4))
        w2p = ctx.enter_context(tc.tile_pool(name="w2s", bufs=4))
        w2bp = ctx.enter_context(tc.tile_pool(name="w2bp", bufs=8))
        outp = ctx.enter_context(tc.tile_pool(name="outp", bufs=2))
        rcp = ctx.enter_context(tc.tile_pool(name="rcp", bufs=1))
        stgp = ctx.enter_context(tc.tile_pool(name="stgp", bufs=2))

        # attention PSUM pools (closed before proj)
        ps_ctx = tc.tile_pool(name="ps", bufs=2, space="PSUM")
        ps = ps_ctx.__enter__()
        sc_ctx = tc.tile_pool(name="scps", bufs=2, space="PSUM")
        scp = sc_ctx.__enter__()
        wv_ctx = tc.tile_pool(name="wvps", bufs=2, space="PSUM")
        wvp = wv_ctx.__enter__()

        a2a_in = [nc.dram_tensor(f"a2a_in{b}", [8 * 128, TSB], BF16,
                                 kind="Internal").ap() for b in range(B)]
        a2a_out = [nc.dram_tensor(f"a2a_out{b}", [8 * 128, TSB], BF16,
                                  kind="Internal").ap() for b in range(B)]
        RG8 = [[0, 1, 2, 3, 4, 5, 6, 7]]

        # ---- gpsimd-cheap setup first: memsets (no DMA deps) ----
        ones1 = st.tile([1, 64], BF16, tag="ones1", name="ones1")
        nc.gpsimd.memset(ones1[:], 1.0)
        mask_big = st.tile([128, 896], BF16, tag="mask", name="mask_big")
        nc.gpsimd.memset(mask_big[:], 1.0)
        nc.gpsimd.affine_select(mask_big[:], mask_big[:], pattern=[[1, 896]],
                                compare_op=ALU.is_ge, fill=0.0, base=-384,
                                channel_multiplier=-1)
        v_sb = [[None] * (T // 128) for _ in range(B)]
        for b in range(B):
            for tk in range(T // 128):
                vt = st.tile([128, 2 * 65], BF16, tag=f"v{b}_{tk}",
                             name=f"v_sb{b}_{tk}")
                nc.gpsimd.memset(vt[:], 1.0)
                v_sb[b][tk] = vt

        # ---- warm-up matmuls: run during the initial input DMA window
        # ---- (mask_big doubles as the warm-up operand) ----
        for wi in range(2):
            acc = ps.tile([128, 512], F32, tag="ps", name=f"wu{wi}")
            for _ in range(18):
                nc.tensor.matmul(acc[:], mask_big[:, 0:128],
                                 mask_big[:, 128:640], start=True, stop=True)

        # ---- input loads: all on the sync queue (hardware DGE) ----
        xt_sb = [[None] * CC for _ in range(B)]
        for b in range(B):
            for cc in range(CC):
                xt_sb[b][cc] = big.tile([128, T], BF16, tag="big",
                                        name=f"xt_sb{b}_{cc}")
        # qkv weights first (small, needed by the very first matmuls),
        # then batch-0 x t-chunk-major so qt=0 work can start early
        wqkv_t = {}
        for nm, d_ in (("k", wk_d), ("q", wq_d), ("v", wv_d)):
            t_ = st.tile([128, C], BF16, tag=f"w{nm}", name=f"w{nm}_t")
            nc.sync.dma_start(t_[:], d_)
            wqkv_t[nm] = t_
        for q4 in range(4):
            for cc in range(CC):
                nc.sync.dma_start(xt_sb[0][cc][:, q4 * 512:(q4 + 1) * 512],
                                  xt_bf[0, cc][:, q4 * 512:(q4 + 1) * 512])
        for cc in range(CC):
            nc.sync.dma_start(xt_sb[1][cc][:], xt_bf[1, cc][:])
        wp_t = st.tile([128, 8 * CC * 128], BF16, tag="wp", name="wp_t")
        nc.sync.dma_start(wp_t[:], wp_d)
        bp_t = st.tile([128, CC], F32, tag="bp", name="bp_t")
        nc.sync.dma_start(bp_t[:], bp_d)
        b1_t = st.tile([128, FB], F32, tag="b1", name="b1_t")
        nc.sync.dma_start(b1_t[:], b1_d)
        b2_t = st.tile([128, CC], F32, tag="b2", name="b2_t")
        nc.sync.dma_start(b2_t[:], b2_d)

        qT = [st.tile([128, T], BF16, tag=f"qT{b}", name=f"qT_sb{b}")
              for b in range(B)]
        kT = [st.tile([128, T], BF16, tag=f"kT{b}", name=f"kT_sb{b}")
              for b in range(B)]
        exq = {}  # (b, qt, hh, b2i) -> exp tile

        # ---------- unit emitters ----------
        def qk_unit(b, nm, tt):
            # one 512-token slice of qT/kT for batch b
            dst = kT[b] if nm == "k" else qT[b]
            acc = ps.tile([128, 512], F32, tag="ps", name=f"pqk_{nm}{b}{tt}")
            w_t = wqkv_t[nm]
            for cc in range(CC):
                nc.tensor.matmul(
                    acc[:],
                    w_t[:, cc * 128:(cc + 1) * 128],
                    xt_sb[b][cc][:, tt * 512:(tt + 1) * 512],
                    start=(cc == 0), stop=(cc == CC - 1),
                )
            nc.vector.tensor_copy(dst[:, tt * 512:(tt + 1) * 512], acc[:])

        def v_unit(b, tk):
            vt = v_sb[b][tk]
            acc = ps.tile([128, 128], F32, tag="ps", name=f"ps_v{b}_{tk}")
            for cc in range(CC):
                nc.tensor.matmul(
                    acc[:],
                    xt_sb[b][cc][:, tk * 128:(tk + 1) * 128],
                    wqkv_t["v"][:, cc * 128:(cc + 1) * 128],
                    start=(cc == 0), stop=(cc == CC - 1),
                )
            src = acc.rearrange("p (h d) -> p h d", h=2)
            dstv = vt.rearrange("p (h d) -> p h d", h=2, d=65)[:, :, 0:64]
            nc.vector.tensor_copy(dstv, src)

        def s_unit(b, qt, b2i):
            # scores + exp (+ causal mask) for k-chunks 2*b2i, 2*b2i+1.
            # The two hh matmuls sit in disjoint PE row groups (partitions
            # 0-63 vs 64-127) so they run concurrently.
            sc = [None, None]
            for hh in range(2):
                sc[hh] = scp.tile([128, 1024], F32, tag="sc",
                                  name=f"psc{b}{hh}{qt}_{b2i}")
            for j in range(2):
                kc = 2 * b2i + j
                for hh in range(2):
                    p0 = 64 * hh
                    nc.tensor.matmul(
                        sc[hh][:, j * 512:(j + 1) * 512],
                        kT[b][p0:p0 + 64, kc * 128:(kc + 1) * 128],
                        qT[b][p0:p0 + 64, qt * 512:(qt + 1) * 512],
                        start=True, stop=True,
                    )
            for hh in range(2):
                ext = expp.tile([128, 1024], BF16, tag="expp",
                                name=f"ex{b}{hh}{qt}_{b2i}")
                nc.scalar.activation(ext[:], sc[hh][:], AF.Exp, scale=SCALE)
                for j in range(2):
                    kc = 2 * b2i + j
                    if kc >= 4 * qt:
                        dd = kc * 128 - qt * 512
                        nc.vector.tensor_mul(
                            ext[:, j * 512:(j + 1) * 512],
                            ext[:, j * 512:(j + 1) * 512],
                            mask_big[:, 384 - dd:896 - dd],
                        )
                exq[(b, qt, hh, b2i)] = ext

        def w_unit(b, qt, b2i, wv_acc):
            nkc = 4 * (qt + 1)
            for j in range(2):
                kc = 2 * b2i + j
                for hh in range(2):
                    ext = exq[(b, qt, hh, b2i)]
                    nc.tensor.matmul(
                        wv_acc[hh][:],
                        v_sb[b][kc][:, hh * 65:hh * 65 + 65],
                        ext[:, j * 512:(j + 1) * 512],
                        start=(kc == 0), stop=(kc == nkc - 1),
                    )

        def f_unit(b, qt, wv_acc):
            # normalize by the softmax denominator (the ones-column of V)
            stg = stgp.tile([128, 512], BF16, tag="stg", name=f"stg{b}{qt}")
            for hh in range(2):
                p0 = 64 * hh
                den = rcp.tile([1, 512], F32, tag="den", name=f"den{b}{hh}{qt}")
                nc.vector.tensor_copy(den[:], wv_acc[hh][64:65, :])
                rc = rcp.tile([1, 512], F32, tag="rc", name=f"rc{b}{hh}{qt}")
                nc.vector.reciprocal_approx_fast(rc[:], den[:])
                rcb = rcp.tile([1, 512], BF16, tag="rcb", name=f"rcb{b}{hh}{qt}")
                nc.vector.tensor_copy(rcb[:], rc[:])
                rb = ps.tile([64, 512], F32, tag="ps", name=f"rb{b}{hh}{qt}")
                nc.tensor.matmul(rb[:], ones1[:], rcb[:], start=True, stop=True)
                rbs = rcp.tile([64, 512], BF16, tag="rbs", name=f"rbs{b}{hh}{qt}")
                nc.vector.tensor_copy(rbs[:], rb[:])
                nc.vector.tensor_mul(stg[p0:p0 + 64, :], wv_acc[hh][0:64, :],
                                     rbs[:])
            # stage the two dest-shard chunks for the AllToAll (gpsimd queue
            # so they don't sit behind weight loads on the sync DMA queue)
            for j in range(2):
                s = 2 * qt + j
                nc.gpsimd.dma_start(
                    a2a_in[b][s * 128:(s + 1) * 128, :],
                    stg[:, j * TSB:(j + 1) * TSB])

        # ---------- attention schedule ----------
        # fillers(b, qt) = prerequisite units of the NEXT qt block, emitted
        # as PE filler between the scalar-gated s/w steps of this block.
        fillers = {
            (0, 0): [("qk", 0, "k", 1), ("qk", 0, "q", 1)] +
                    [("v", 0, tk) for tk in range(4, 8)],
            (0, 1): [("qk", 0, "k", 2), ("qk", 0, "q", 2)] +
                    [("v", 0, tk) for tk in range(8, 12)],
            (0, 2): [("qk", 0, "k", 3), ("qk", 0, "q", 3)] +
                    [("v", 0, tk) for tk in range(12, 16)],
            (0, 3): [("qk", 1, "k", 0), ("qk", 1, "q", 0)] +
                    [("v", 1, tk) for tk in range(0, 4)],
            (1, 0): [("qk", 1, "k", 1), ("qk", 1, "q", 1)] +
                    [("v", 1, tk) for tk in range(4, 8)],
            (1, 1): [("qk", 1, "k", 2), ("qk", 1, "q", 2)] +
                    [("v", 1, tk) for tk in range(8, 12)],
            (1, 2): [("qk", 1, "q", 3), ("v", 1, 12), ("v", 1, 13)],
            (1, 3): [("qk", 1, "k", 3), ("v", 1, 14), ("v", 1, 15)],
        }

        def emit_filler(u):
            if u[0] == "qk":
                qk_unit(u[1], u[2], u[3])
            else:
                v_unit(u[1], u[2])

        def qt_block(b, qt, fin, post=None):
            # `fin` finalizes the PREVIOUS qt: its rb broadcast matmuls wait
            # on a short DVE chain, so they are emitted after this block's
            # first scores unit to keep the PE fed across the boundary.
            nb2i = 2 * (qt + 1)
            fl = list(fillers[(b, qt)])
            # for (1,3) the fillers are this block's OWN late prerequisites:
            # kT(1,3) is only needed from b2i=6, v(1,14/15) from b2i=7.
            own_late = (b, qt) == (1, 3)
            wv_acc = [wvp.tile([65, 512], F32, tag="wv",
                               name=f"pwv{b}{hh}{qt}")
                      for hh in range(2)]
            for i in range(nb2i):
                if own_late:
                    if i == 2 and fl:
                        emit_filler(fl.pop(0))       # kT(1,3)
                    if i == 4 and len(fl) == 2:
                        emit_filler(fl.pop(0))       # v(1,14)
                        emit_filler(fl.pop(0))       # v(1,15)
                s_unit(b, qt, i)
                if i == 0 and fin is not None:
                    fin()
                if i >= 1:
                    w_unit(b, qt, i - 1, wv_acc)
                if not own_late:
                    if fl:
                        emit_filler(fl.pop(0))
                    if fl and len(fl) > nb2i - 1 - i:
                        emit_filler(fl.pop(0))
            while fl:
                emit_filler(fl.pop(0))
            w_unit(b, qt, nb2i - 1, wv_acc)
            if post is not None:
                post()
            return lambda: f_unit(b, qt, wv_acc)

        # batch 0 prerequisites
        qk_unit(0, "k", 0)
        qk_unit(0, "q", 0)
        for tk in range(4):
            v_unit(0, tk)

        fin = None
        for qt in range(4):
            fin = qt_block(0, qt, fin)
        # qt_block(1,0) flushes f(0,3) at its head, so the batch-0 staging
        # DMAs precede the first collective on the gpsimd queue
        fin = qt_block(1, 0, fin)

        nc.gpsimd.collective_compute(
            "AllToAll", ALU.bypass, replica_groups=RG8,
            ins=[a2a_in[0]], outs=[a2a_out[0]],
        )
        rt_sb = [st.tile([128, TS], BF16, tag=f"rt{s}", name=f"rt{s}")
                 for s in range(8)]

        def emit_rcv0():
            # batch-0 receive DMAs; emitted after stg(1,1) so their wait on
            # the first AllToAll never blocks batch-1 staging on gpsimd
            for s in range(8):
                nc.gpsimd.dma_start(rt_sb[s][:, 0:TSB],
                                    a2a_out[0][s * 128:(s + 1) * 128, :])

        def emit_w1_loads():
            # stream during batch-1 attention on the sync queue
            for half in range(2):
                for cc in range(CC):
                    t_ = big.tile([128, 2048], BF16, tag="big",
                                  name=f"w1_sb{cc}_{half}")
                    nc.sync.dma_start(
                        t_[:], w1_d[cc][:, half * 2048:(half + 1) * 2048])
                    w1_sb[cc][half] = t_

        w1_sb = [[None, None] for _ in range(CC)]

        def post_11():
            emit_rcv0()
            emit_w1_loads()
            # xs (fp32 residual): right behind w1 on the sync queue, done
            # well before the proj residual-adds need it
            for xi in range(4):
                xst = big.tile([128, 1024], F32, tag="xsb", bufs=4,
                               name=f"xs{xi}")
                nc.sync.dma_start(xst[:], xs_d[:, xi * 1024:(xi + 1) * 1024])
                xs_sb.append(xst)

        xs_sb = []
        fin = qt_block(1, 1, fin, post=post_11)
        fin = qt_block(1, 2, fin)
        fin = qt_block(1, 3, fin)
        fin()  # f(1,3) immediately: the second AllToAll depends on it

        wv_ctx.__exit__(None, None, None)
        sc_ctx.__exit__(None, None, None)
        ps_ctx.__exit__(None, None, None)

        # ---- proj (streamed): per cb, 8 matmuls into a ring-2 PSUM tile,
        # residual-add immediately, release. Batch-0 columns only need the
        # first exchange; together with the batch-0 halves of the first NFH
        # FFN1 row-blocks they keep the PE busy for the whole span of the
        # batch-1 AllToAll.
        pf1_ctx = tc.tile_pool(name="pf1", bufs=2, space="PSUM")
        pf1 = pf1_ctx.__enter__()
        pjp_ctx = tc.tile_pool(name="pjp", bufs=2, space="PSUM")
        pjp = pjp_ctx.__enter__()

        def xs_slice(cb):
            return xs_sb[cb // 2][:, (cb % 2) * 512:(cb % 2) * 512 + 512]

        x1b = [st.tile([128, TS], BF16, tag=f"x1b{cb}", name=f"x1b{cb}")
               for cb in range(CC)]

        def proj_cb(cb, h0):
            acc = pjp.tile([128, TSB], F32, tag="pj", name=f"ps_pj{cb}_{h0}")
            for s in range(8):
                nc.tensor.matmul(
                    acc[:],
                    wp_t[:, (s * CC + cb) * 128:(s * CC + cb + 1) * 128],
                    rt_sb[s][:, h0 * TSB:(h0 + 1) * TSB],
                    start=(s == 0), stop=(s == 7),
                )
            nc.vector.scalar_tensor_tensor(
                x1b[cb][:, h0 * TSB:(h0 + 1) * TSB], acc[:],
                bp_t[:, cb:cb + 1],
                xs_slice(cb)[:, h0 * TSB:(h0 + 1) * TSB], ALU.add, ALU.add)

        hT = [st.tile([128, TS], BF16, tag=f"hT{fb}", name=f"hT{fb}")
              for fb in range(FB)]
        NFH = 12  # FFN1 row-blocks computed in batch-halves around the a2a

        def ffn1_half(fb, h0):
            w1h, fo = fb // 16, fb % 16
            acc = pf1.tile([128, TSB], F32, tag="pf1", name=f"ps_h{fb}_{h0}")
            for cc in range(CC):
                nc.tensor.matmul(
                    acc[:],
                    w1_sb[cc][w1h][:, fo * 128:(fo + 1) * 128],
                    x1b[cc][:, h0 * TSB:(h0 + 1) * TSB],
                    start=(cc == 0), stop=(cc == CC - 1))
            nc.scalar.activation(hT[fb][:, h0 * TSB:(h0 + 1) * TSB], acc[:],
                                 AF.Relu, bias=b1_t[:, fb:fb + 1])

        for cb in range(CC):
            proj_cb(cb, 0)

        nc.gpsimd.collective_compute(
            "AllToAll", ALU.bypass, replica_groups=RG8,
            ins=[a2a_in[1]], outs=[a2a_out[1]],
        )
        for s in range(8):
            nc.gpsimd.dma_start(rt_sb[s][:, TSB:TS],
                                a2a_out[1][s * 128:(s + 1) * 128, :])

        for fb in range(NFH):
            ffn1_half(fb, 0)

        for cb in range(CC):
            proj_cb(cb, 1)
        pjp_ctx.__exit__(None, None, None)

        # ---- FFN1 (remaining) with FFN2 group A (cb 0..NA-1) interleaved
        pfa_ctx = tc.tile_pool(name="pfa", bufs=NA, space="PSUM")
        pfa = pfa_ctx.__enter__()
        accA = [pfa.tile([128, TS], F32, tag=f"pfa{cb}", bufs=1,
                         name=f"ps_oa{cb}") for cb in range(NA)]
        w2b_sb = []

        def ffn2a_mms(fb):
            wt = w2p.tile([128, NA * 128], BF16, tag="w2s", name=f"w2ta{fb}")
            nc.sync.dma_start(wt[:], w2_d[fb][:, 0:NA * 128])
            if fb >= FB - 8:
                # prefetch the first 8 group-B w2 slices during the FFN1 tail
                wtb = w2bp.tile([128, (CC - NA) * 128], BF16, tag="w2b",
                                name=f"w2tb{fb - (FB - 8)}")
                nc.sync.dma_start(wtb[:], w2_d[fb - (FB - 8)][:, NA * 128:C])
                w2b_sb.append(wtb)
            for cb in range(NA):
                nc.tensor.matmul(
                    accA[cb][:], wt[:, cb * 128:(cb + 1) * 128], hT[fb][:],
                    start=(fb == 0), stop=(fb == FB - 1))

        for fb in range(NFH):
            ffn1_half(fb, 1)
            ffn2a_mms(fb)
        for fb in range(NFH, FB):
            w1h, fo = fb // 16, fb % 16
            acc = pf1.tile([128, TS], F32, tag="pf1", name=f"ps_h{fb}")
            for cc in range(CC):
                nc.tensor.matmul(
                    acc[:],
                    w1_sb[cc][w1h][:, fo * 128:(fo + 1) * 128],
                    x1b[cc][:],
                    start=(cc == 0), stop=(cc == CC - 1))
            nc.scalar.activation(hT[fb][:], acc[:], AF.Relu,
                                 bias=b1_t[:, fb:fb + 1])
            ffn2a_mms(fb)

        # ---- FFN2 group B (cb NA..7): reuses the pf1 ring slots; w2
        # slices beyond the prefetched 8 stream just-in-time ----
        accB = [pf1.tile([128, TS], F32, tag="pf1", name=f"ps_ob{cb}")
                for cb in range(CC - NA)]
        first = True
        for fc in range(FB):
            wt = w2b_sb[fc]
            for cb in range(CC - NA):
                nc.tensor.matmul(
                    accB[cb][:], wt[:, cb * 128:(cb + 1) * 128], hT[fc][:],
                    start=(fc == 0), stop=(fc == FB - 1))
            if fc + 8 < FB:
                wtb = w2bp.tile([128, (CC - NA) * 128], BF16, tag="w2b",
                                name=f"w2tb{fc + 8}")
                nc.sync.dma_start(wtb[:], w2_d[fc + 8][:, NA * 128:C])
                w2b_sb.append(wtb)
            if first:
                first = False
                # group-A outputs drain while B accumulates
                for cb in range(NA):
                    ot = outp.tile([128, TS], F32, tag="outp", name=f"ot{cb}")
                    nc.vector.scalar_tensor_tensor(
                        ot[:], accA[cb][:], b2_t[:, cb:cb + 1],
                        x1b[cb][:], ALU.add, ALU.add)
                    nc.sync.dma_start(out_d[cb], ot[:])
        for cb4 in range(CC - NA):
            cb = cb4 + NA
            ot = outp.tile([128, TS], F32, tag="outp", name=f"ot{cb}")
            nc.vector.scalar_tensor_tensor(ot[:], accB[cb4][:],
                                           b2_t[:, cb:cb + 1],
                                           x1b[cb][:], ALU.add, ALU.add)
            nc.sync.dma_start(out_d[cb], ot[:])

        pfa_ctx.__exit__(None, None, None)
        pf1_ctx.__exit__(None, None, None)


_CACHED = None


def _get_compiled():
    global _CACHED
    if _CACHED is None:
        nc = bacc.Bacc("TRN2", target_bir_lowering=False, debug=False,
                       num_devices=N_CORES)
        build_program(nc)
        nc.compile()
        _CACHED = nc
    return _CACHED


def _prep_inputs(x, Wq, Wk, Wv, Wproj, bproj, W1, b1, W2, b2):
    bf = ml_dtypes.bfloat16
    W1t = np.ascontiguousarray(W1.astype(bf).reshape(CC, 128, F))
    W2t = np.ascontiguousarray(W2.astype(bf).reshape(FB, 128, C))
    b1r = np.ascontiguousarray(b1.astype(np.float32).reshape(FB, 128).T)
    b2r = np.ascontiguousarray(b2.astype(np.float32).reshape(CC, 128).T)
    bpr = np.ascontiguousarray(bproj.astype(np.float32).reshape(CC, 128).T)
    # full Wproj on every core: wp_flat[p, (s*CC+cb)*128+k] = Wproj[128s+p, 128cb+k]
    wp_flat = np.ascontiguousarray(
        Wproj.astype(bf).reshape(8, 128, CC, 128).transpose(1, 0, 2, 3)
        .reshape(128, 8 * CC * 128))
    # x transposed, both batches, shared by all cores
    xT = [np.ascontiguousarray(x[b].T.astype(np.float32)) for b in range(B)]
    xT_bf = np.ascontiguousarray(
        np.stack([xT[b].astype(bf).reshape(CC, 128, T) for b in range(B)]))

    in_maps = []
    for c in range(N_CORES):
        cols = slice(128 * c, 128 * (c + 1))
        # wq_t[p, cc*128+k] = Wq[128cc+p, core_cols[k]]
        wq_s = np.ascontiguousarray(
            Wq[:, cols].astype(bf).reshape(CC, 128, 128)
            .transpose(1, 0, 2).reshape(128, C))
        wk_s = np.ascontiguousarray(
            Wk[:, cols].astype(bf).reshape(CC, 128, 128)
            .transpose(1, 0, 2).reshape(128, C))
        wv_s = np.ascontiguousarray(
            Wv[:, cols].astype(bf).reshape(CC, 128, 128)
            .transpose(1, 0, 2).reshape(128, C))
        tok = slice(TSB * c, TSB * (c + 1))
        # xs_t[p, cb*512+j] = fused-shard residual, fp32
        xts = np.ascontiguousarray(
            np.concatenate([xT[0][:, tok], xT[1][:, tok]], axis=1)
            .reshape(CC, 128, TS).transpose(1, 0, 2).reshape(128, CC * TS))
        in_maps.append({
            "xt_bf": xT_bf, "xs": xts,
            "wq": wq_s, "wk": wk_s, "wv": wv_s, "wp": wp_flat,
            "w1": W1t, "w2": W2t, "bp": bpr, "b1": b1r, "b2": b2r,
        })
    return in_maps


def kernel(x, Wq, Wk, Wv, Wproj, bproj, W1, b1, W2, b2, _trace=False):
    nc = _get_compiled()
    in_maps = _prep_inputs(np.asarray(x), np.asarray(Wq), np.asarray(Wk),
                           np.asarray(Wv), np.asarray(Wproj), np.asarray(bproj),
                           np.asarray(W1), np.asarray(b1), np.asarray(W2),
                           np.asarray(b2))
    res = run_bass_kernel_spmd(nc, in_maps, list(range(N_CORES)), trace=_trace)
    out = np.empty((B, T, C), dtype=np.float32)
    for c in range(N_CORES):
        shard = res.results[c]["outT"].reshape(C, TS)
        for b in range(B):
            out[b, TSB * c: TSB * (c + 1), :] = shard[:, TSB * b:TSB * (b + 1)].T
    if _trace:
        kernel.last_exec_time_ns = res.exec_time_ns
    return out
